# revision 3
# baseline (speedup 1.0000x reference)
"""Deformable-attention-3D Trainium2 kernel, v2 (bf16 datapath).

Sharding: 8 cores = (batch b in {0,1}) x (query-block j in {0..3}).
Each core: NB=8192 queries of one batch, full xT slab with halo, bf16.

Structure per core:
  P2a+P1 (interleaved, single x pass): offsets/attn logits [96, n] with
      DMA-repack into packed [128 = 4r x 32hp, 2048] tiles (repack DMAs on
      the ACT hwdge queue), and value projection V[g] bf16 [128, XEXT].
  P2b: softmax (exp on ACT, p-sum via block-diag selector matmul, recip on
      DVE); tents t = (|u|-1) min 0 = -relu(1-|u|) via ACT Abs (bias folded)
      + DVE tensor_scalar min; both tents negated so products are positive.
  P3: 21-cell tent MAC (corners dropped) in 3 waves grouped by sy.
      Per cell: wrep = sel^T @ pr (PE, K=32, 512-wide matmuls - ISA caps
      matmul moving/dst at one PSUM bank); ACT copies wrep to bf16 SBUF;
      V (x) wrep on DVE or Pool (split tuned to measured HW rates: DVE TT
      bf16 ~1.7us/[128,2048], Pool ~4.2us, ACT copy ~0.61us/[128,512]);
      cell-sum via identity-matmul PSUM accumulation on PE (deferred by one
      cell so the in-order PE never stalls), spilled per (chunk,g) into
      accS (SBUF bf16) by DVE PSUM-direct adds; cell 0 of each chunk
      avoids PSUM so the previous chunk's spill slots in race-free.
  P4: output projection interleaved into the last wave (bf16 matmuls,
      bout via ACT bias), bf16 out staged through SBUF, host upcasts.
"""
import numpy as np
from contextlib import ExitStack

import concourse.bass as bass
import concourse.mybir as mybir
import concourse.tile as tile

F32 = mybir.dt.float32
BF16 = mybir.dt.bfloat16
AF = mybir.ActivationFunctionType
OP = mybir.AluOpType

# problem constants
B, C, Z, HH, WW = 2, 256, 16, 32, 64
H, W = HH * Z, WW          # 512, 64
N = H * W                  # 32768 queries per batch
HEADS, P, DH = 8, 4, 32
NB = N // 4                # 8192 queries per core
HALO = 192
XEXT = NB + 2 * HALO       # 8576
CK = 1024                  # chunk (queries per cell-loop tile)
NCH = NB // CK             # 8 chunks
RB = 2048                  # r-block width (queries per packed partition group)

SYS = (-2, -1, 0, 1, 2)
SXS = (-2, -1, 0, 1, 2)
XMI = {-2: 0, -1: 1, 1: 2, 2: 3}

# waves of sy rows; cells (sy, sx) drop the 4 corners
WAVES = [(-2, -1), (0,), (1, 2)]


def _wave_cells(wave):
    cells = []
    for sy in wave:
        for sx in SXS:
            if abs(sy) == 2 and abs(sx) == 2:
                continue
            cells.append((sy, sx))
    return cells


# per-cell tuning: mult mode ('act' = ACT copies wrep to bf16 SBUF so the DVE
# mult runs 2x; 'dir' = DVE reads wrep from PSUM at 1x) and add target
# ('pe' = identity-matmul into PSUM accP; 'pool'/'dve' = TT add into accS;
# 'w0' = first contribution, mult writes accS directly).
# Cell 0 must not touch accP: its slot is where the previous chunk's accP
# spill is emitted (before accP reallocation), keeping engines fed without
# a PSUM write-after-read race.
MODE_TAB = [
    {0: ('act', 'w0'), 2: ('dir', 'pe'), 3: ('actpool', 'pe'),
     5: ('actpool', 'pe'), 7: ('actpool', 'pe')},
    {0: ('act', 'dve'), 2: ('actpool', 'pe'), 4: ('actpool', 'pe')},
    {0: ('act', 'dve'), 2: ('actpool', 'pe'), 4: ('actpool', 'pe'),
     6: ('actpool', 'pe')},
]


def _cell_modes(wi, cells):
    return {c: MODE_TAB[wi].get(i, ('act', 'pe')) for i, c in enumerate(cells)}


_cache = {}


def build_program(reps=1):
    key = ("nc", reps)
    if key in _cache:
        return _cache[key]
    nc = bass.Bass()
    d = {}
    d["xT"] = nc.dram_tensor("xT", [C, XEXT], BF16, kind="ExternalInput").ap()
    d["wv"] = nc.dram_tensor("wv", [C, C], BF16, kind="ExternalInput").ap()
    d["woa"] = nc.dram_tensor("woa", [C, 96], BF16, kind="ExternalInput").ap()
    d["wo"] = nc.dram_tensor("wo", [C, C], BF16, kind="ExternalInput").ap()
    d["sel"] = nc.dram_tensor("sel", [128, 512], BF16, kind="ExternalInput").ap()
    d["selp"] = nc.dram_tensor("selp", [128, 128], BF16, kind="ExternalInput").ap()
    d["iden"] = nc.dram_tensor("iden", [128, 128], BF16, kind="ExternalInput").ap()
    d["bval"] = nc.dram_tensor("bval", [C, 1], F32, kind="ExternalInput").ap()
    d["bout"] = nc.dram_tensor("bout", [C, 1], F32, kind="ExternalInput").ap()
    d["xmask"] = nc.dram_tensor("xmask", [128, 2 * RB], BF16, kind="ExternalInput").ap()
    d["vmask"] = nc.dram_tensor("vmask", [128, 2 * HALO], BF16, kind="ExternalInput").ap()
    d["biasy"] = nc.dram_tensor("biasy", [128, 5], F32, kind="ExternalInput").ap()
    d["biasx"] = nc.dram_tensor("biasx", [128, 5], F32, kind="ExternalInput").ap()
    d["boaa"] = nc.dram_tensor("boaa", [128, 1], F32, kind="ExternalInput").ap()
    d["out"] = nc.dram_tensor("out", [C, NB], BF16, kind="ExternalOutput").ap()
    with tile.TileContext(nc) as tc, ExitStack() as ctx:
        if reps == 1:
            _kernel_body(ctx, tc, d)
        else:
            with tc.For_i(0, reps, 1):
                _kernel_body(ctx, tc, d)
    _legalize_waits(nc)
    _cache[key] = nc
    return nc


def _legalize_waits(nc):
    """Split >1 sem-waits onto standalone EventSemaphore instructions (this
    toolchain's walrus rejects multi-wait on HW-decoded structs)."""
    nid = 0
    for f in nc.m.functions:
        for bb in f.blocks:
            insts = bb.instructions
            i = 0
            while i < len(insts):
                inst = insts[i]
                si = inst.sync_info
                waits = list(si.on_wait) if (si and si.on_wait) else []
                if len(waits) > 1:
                    keep = waits[-1:]
                    excess = waits[:-1]
                    inst.sync_info = mybir.SyncInfo(
                        on_wait=keep, on_update=list(si.on_update or []))
                    for w in excess:
                        ws = mybir.InstEventSemaphore(
                            name=f"WSPLIT-{nid}", ins=[], outs=[],
                            sync_info=mybir.SyncInfo(on_wait=[w], on_update=[]))
                        nid += 1
                        ws.engine = inst.engine
                        nc.register_instruction(ws, overwrite=True)
                        insts.insert(i, ws)
                        i += 1
                i += 1


def _kernel_body(ctx, tc, d):
    nc = tc.nc
    const = ctx.enter_context(tc.tile_pool(name="const", bufs=1))
    xin = ctx.enter_context(tc.tile_pool(name="xin", bufs=2))
    vpool = ctx.enter_context(tc.tile_pool(name="vpool", bufs=1))
    rawp = ctx.enter_context(tc.tile_pool(name="rawp", bufs=1))
    tentp = ctx.enter_context(tc.tile_pool(name="tentp", bufs=1))
    prp = ctx.enter_context(tc.tile_pool(name="prp", bufs=1))
    wbp = ctx.enter_context(tc.tile_pool(name="wbp", bufs=6))
    tmpp = ctx.enter_context(tc.tile_pool(name="tmpp", bufs=7))
    accs = ctx.enter_context(tc.tile_pool(name="accs", bufs=1))
    psW = ctx.enter_context(tc.tile_pool(name="psW", bufs=3, space="PSUM"))
    psA = ctx.enter_context(tc.tile_pool(name="psA", bufs=1, space="PSUM"))

    # ---- constants to SBUF
    wv_t = [const.tile([128, C], BF16, tag=f"wv{k}", name=f"wv{k}") for k in range(2)]
    woa_t = [const.tile([128, 96], BF16, tag=f"woa{k}", name=f"woa{k}") for k in range(2)]
    wo_t = [const.tile([128, C], BF16, tag=f"wo{k}", name=f"wo{k}") for k in range(2)]
    sel_t = const.tile([128, 512], BF16, tag="sel", name="sel")
    selp_t = const.tile([128, 128], BF16, tag="selp", name="selp")
    iden_t = const.tile([128, 128], BF16, tag="iden", name="iden")
    bval_t = [const.tile([128, 1], F32, tag=f"bv{g}", name=f"bv{g}") for g in range(2)]
    bout_t = [const.tile([128, 1], F32, tag=f"bo{m}", name=f"bo{m}") for m in range(2)]
    xm_t = const.tile([128, 2 * RB], BF16, tag="xm", name="xm")
    vm_t = const.tile([128, 2 * HALO], BF16, tag="vm", name="vm")
    biasy_t = const.tile([128, 5], F32, tag="biasy", name="biasy")
    biasx_t = const.tile([128, 5], F32, tag="biasx", name="biasx")
    boaa_t = const.tile([128, 1], F32, tag="boaa", name="boaa")
    # consts needed by the first chunks go on SP ahead of the x-loads; the
    # bulkier late-use consts (sel/xm/selp/iden) ride the ACT hwdge queue.
    for k in range(2):
        nc.sync.dma_start(wv_t[k][:], d["wv"][k * 128:(k + 1) * 128, :])
        nc.sync.dma_start(woa_t[k][:], d["woa"][k * 128:(k + 1) * 128, :])
    for g in range(2):
        nc.sync.dma_start(bval_t[g][:], d["bval"][g * 128:(g + 1) * 128, :])
        nc.sync.dma_start(bout_t[g][:], d["bout"][g * 128:(g + 1) * 128, :])
    nc.sync.dma_start(biasy_t[:], d["biasy"][:])
    nc.sync.dma_start(biasx_t[:], d["biasx"][:])
    nc.sync.dma_start(boaa_t[:], d["boaa"][:])
    for k in range(2):
        nc.scalar.dma_start(wo_t[k][:], d["wo"][k * 128:(k + 1) * 128, :])
    nc.scalar.dma_start(sel_t[:], d["sel"][:])
    nc.scalar.dma_start(selp_t[:], d["selp"][:])
    nc.scalar.dma_start(iden_t[:], d["iden"][:])
    nc.scalar.dma_start(xm_t[:], d["xmask"][:])
    nc.scalar.dma_start(vm_t[:], d["vmask"][:])

    # ---- P2a + P1 interleaved, single x pass: chunk ch covers columns
    # [HALO + ch*CK, HALO + (ch+1)*CK) of xT, feeding both the offsets/attn
    # matmul (repacked via the ACT hwdge queue) and the value projection.
    # Two small edge chunks fill V's halo columns.
    oyp = rawp.tile([128, RB], BF16, tag="oyp", name="oyp")
    oxp = rawp.tile([128, RB], BF16, tag="oxp", name="oxp")
    elp = rawp.tile([128, RB], BF16, tag="elp", name="elp")
    V = [vpool.tile([128, XEXT], BF16, tag=f"V{g}", name=f"V{g}") for g in range(2)]

    def vproj(xt, c0, f):
        for g in range(2):
            pv = psW.tile([128, CK], F32, tag="w", name="pv")
            for h in range(0, f, 512):
                hw = min(512, f - h)
                for k in range(2):
                    nc.tensor.matmul(pv[:, h:h + hw],
                                     wv_t[k][:, g * 128:(g + 1) * 128],
                                     xt[k][:, h:h + hw],
                                     start=(k == 0), stop=(k == 1))
            nc.scalar.activation(V[g][:, c0:c0 + f], pv[:, 0:f],
                                 AF.Identity, bias=bval_t[g][:], scale=1.0)

    for ch in range(NCH):
        r, j0 = ch // 2, (ch % 2) * CK
        xt = [xin.tile([128, CK], BF16, tag=f"xa{k}", name=f"xa{k}")
              for k in range(2)]
        for k in range(2):
            nc.sync.dma_start(
                xt[k][:], d["xT"][k * 128:(k + 1) * 128,
                                  HALO + ch * CK:HALO + (ch + 1) * CK])
        po = psW.tile([128, CK], F32, tag="w", name="po")
        for h in range(0, CK, 512):
            for k in range(2):
                nc.tensor.matmul(po[0:96, h:h + 512], woa_t[k][:],
                                 xt[k][:, h:h + 512],
                                 start=(k == 0), stop=(k == 1))
        rp = wbp.tile([128, CK], BF16, tag="rpk", name="rpk", bufs=2)
        nc.scalar.copy(rp[0:96, :], po[0:96, :])
        for q, dst in enumerate((oyp, oxp, elp)):
            nc.scalar.dma_start(dst[32 * r:32 * (r + 1), j0:j0 + CK],
                                rp[32 * q:32 * (q + 1), :])
        vproj(xt, HALO + ch * CK, CK)
    for c0 in (0, XEXT - HALO):
        xt = [xin.tile([128, CK], BF16, tag=f"xa{k}", name=f"xa{k}")
              for k in range(2)]
        for k in range(2):
            nc.sync.dma_start(xt[k][:, 0:HALO],
                              d["xT"][k * 128:(k + 1) * 128, c0:c0 + HALO])
        vproj(xt, c0, HALO)
    # zero out-of-batch halo
    for g in range(2):
        nc.vector.tensor_tensor(V[g][:, 0:HALO], V[g][:, 0:HALO],
                                vm_t[:, 0:HALO], OP.mult)
        nc.vector.tensor_tensor(V[g][:, XEXT - HALO:XEXT],
                                V[g][:, XEXT - HALO:XEXT],
                                vm_t[:, HALO:2 * HALO], OP.mult)

    # ---- P2b: softmax + x-tents
    ex = rawp.tile([128, RB], BF16, tag="ex", name="ex")
    nc.scalar.activation(ex[:], elp[:], AF.Exp, bias=boaa_t[:], scale=1.0)
    rc = [rawp.tile([128, CK], F32, tag=f"rc{i}", name=f"rc{i}") for i in range(2)]
    for i in range(2):
        pp = psA.tile([128, CK], F32, tag="a", name="pp")
        for h in range(0, CK, 512):
            nc.tensor.matmul(pp[:, h:h + 512], selp_t[:],
                             ex[:, i * CK + h:i * CK + h + 512],
                             start=True, stop=True)
        nc.vector.reciprocal(rc[i][:], pp[:])
    at = rawp.tile([128, RB], BF16, tag="at", name="at")
    for i in range(2):
        nc.vector.tensor_tensor(at[:, i * CK:(i + 1) * CK],
                                ex[:, i * CK:(i + 1) * CK], rc[i][:], OP.mult)

    axm = {}
    for sx in SXS:
        k = sx + 2
        u = tmpp.tile([128, RB], BF16, tag="tu", name="tu", bufs=2)
        nc.scalar.activation(u[:], oxp[:], AF.Abs, bias=biasx_t[:, k:k + 1],
                             scale=1.0)
        t = tentp.tile([128, RB], BF16, tag=f"axm{sx}", name=f"axm{sx}")
        nc.vector.tensor_scalar(t[:], u[:], 1.0, 0.0, OP.subtract, OP.min)
        if sx > 0:
            nc.gpsimd.tensor_tensor(t[:], t[:], xm_t[:, 0:RB], OP.mult)
            if sx == 2:
                nc.gpsimd.tensor_tensor(t[:], t[:], xm_t[:, 1:RB + 1], OP.mult)
        elif sx < 0:
            nc.gpsimd.tensor_tensor(t[:], t[:], xm_t[:, RB:2 * RB], OP.mult)
            if sx == -2:
                nc.gpsimd.tensor_tensor(t[:], t[:], xm_t[:, RB - 1:2 * RB - 1],
                                        OP.mult)
        axm[sx] = t

    accS = [accs.tile([128, NB], BF16, tag=f"s{g}", name=f"s{g}") for g in range(2)]

    # ---- P3: cell-loop waves
    for wi, wave in enumerate(WAVES):
        cells = _wave_cells(wave)
        modes = _cell_modes(wi, cells)
        aya = {}
        for si, sy in enumerate(wave):
            k = sy + 2
            u = tmpp.tile([128, RB], BF16, tag="tu", name="tu", bufs=2)
            nc.scalar.activation(u[:], oyp[:], AF.Abs,
                                 bias=biasy_t[:, k:k + 1], scale=1.0)
            nc.vector.tensor_scalar(u[:], u[:], 1.0, 0.0, OP.subtract, OP.min)
            a = tentp.tile([128, RB], BF16, tag=f"aya{si}", name=f"aya{si}")
            nc.vector.tensor_tensor(a[:], u[:], at[:], OP.mult)
            aya[sy] = a
        pr = {}
        recycle = ("oxp", "elp", "ex")
        for i, (sy, sx) in enumerate(cells):
            if i < 5:
                p = prp.tile([128, RB], BF16, tag=f"pr{i}", name=f"pr{i}")
            else:
                p = rawp.tile([128, RB], BF16, tag=recycle[i - 5],
                              name=f"pr{i}")
            peng = nc.vector if wi == 0 else nc.gpsimd
            peng.tensor_tensor(p[:], aya[sy][:], axm[sx][:], OP.mult)
            pr[(sy, sx)] = p

        pe_cells = [c for c in cells if modes[c][1] == 'pe']

        def spill(ch, g, accP):
            csl = slice(ch * CK, (ch + 1) * CK)
            nc.vector.tensor_tensor(accS[g][:, csl], accS[g][:, csl],
                                    accP[:], OP.add)
            if wi == len(WAVES) - 1 and g == 1:
                for mc in range(2):
                    op = psW.tile([128, CK], F32, tag="w", name="op")
                    for h in range(0, CK, 512):
                        for gg in range(2):
                            nc.tensor.matmul(
                                op[:, h:h + 512],
                                wo_t[gg][:, mc * 128:(mc + 1) * 128],
                                accS[gg][:, ch * CK + h:ch * CK + h + 512],
                                start=(gg == 0), stop=(gg == 1))
                    ob = wbp.tile([128, CK], BF16, tag="ob", name="ob",
                                  bufs=2)
                    nc.scalar.activation(ob[:], op[:], AF.Identity,
                                         bias=bout_t[mc][:], scale=1.0)
                    nc.scalar.dma_start(
                        d["out"][mc * 128:(mc + 1) * 128, csl], ob[:])

        # Per (chunk, g) pass: cell0 avoids accP (spill slot); the identity
        # accumulate of each PE cell is deferred by one cell so the in-order
        # PE never stalls on the DVE mult it consumes.
        pending = None
        for ch in range(NCH):
            r, j0 = ch // 2, (ch % 2) * CK
            csl = slice(ch * CK, (ch + 1) * CK)
            for g in range(2):
                accP = None
                deferred = None
                for ci, cell in enumerate(cells):
                    sy, sx = cell
                    mult, add = modes[cell]
                    vb = HALO + ch * CK + sy * W + sx
                    wr = psW.tile([128, CK], F32, tag="w", name="wr")
                    if r < 3:
                        lhs = sel_t[32 * r:32 * (r + 1), g * 128:(g + 1) * 128]
                        rh = pr[cell][32 * r:32 * (r + 1), j0:j0 + CK]
                    else:
                        lhs = sel_t[64:128, 256 + g * 128:256 + (g + 1) * 128]
                        rh = pr[cell][64:128, j0:j0 + CK]
                    for h in range(0, CK, 512):
                        nc.tensor.matmul(wr[:, h:h + 512], lhs,
                                         rh[:, h:h + 512],
                                         start=True, stop=True)
                    if deferred is not None:
                        deferred()
                        deferred = None
                    vs = V[g][:, vb:vb + CK]
                    if mult in ('act', 'actpool'):
                        wb = wbp.tile([128, CK], BF16, tag="wb", name="wb")
                        nc.scalar.copy(wb[:], wr[:])
                        src = wb
                    else:
                        src = wr
                    meng = nc.gpsimd if mult == 'actpool' else nc.vector
                    if ci == 0:
                        if add == 'w0':
                            nc.vector.tensor_tensor(accS[g][:, csl], src[:],
                                                    vs, OP.mult)
                        else:
                            tm = tmpp.tile([128, CK], BF16, tag="tm",
                                           name="tm")
                            nc.vector.tensor_tensor(tm[:], src[:], vs,
                                                    OP.mult)
                            nc.vector.tensor_tensor(accS[g][:, csl],
                                                    accS[g][:, csl],
                                                    tm[:], OP.add)
                        if pending is not None:
                            spill(*pending)
                            pending = None
                        accP = psA.tile([128, CK], F32, tag="a", name="accP")
                        continue
                    tm = tmpp.tile([128, CK], BF16, tag="tm", name="tm")
                    meng.tensor_tensor(tm[:], src[:], vs, OP.mult)
                    if add == 'dve':
                        nc.vector.tensor_tensor(accS[g][:, csl],
                                                accS[g][:, csl],
                                                tm[:], OP.add)
                    elif add == 'pool':
                        nc.gpsimd.tensor_tensor(accS[g][:, csl],
                                                accS[g][:, csl],
                                                tm[:], OP.add)
                    else:
                        def deferred(tm=tm, accP=accP,
                                     st=(cell == pe_cells[0]),
                                     sp=(cell == pe_cells[-1])):
                            for h in range(0, CK, 512):
                                nc.tensor.matmul(accP[:, h:h + 512],
                                                 iden_t[:], tm[:, h:h + 512],
                                                 start=st, stop=sp)
                if deferred is not None:
                    deferred()
                pending = (ch, g, accP)
        spill(*pending)


def _consts():
    import ml_dtypes
    bf = ml_dtypes.bfloat16
    sel = np.zeros((128, 512), np.float32)
    s32 = np.zeros((2, 32, 128), np.float32)
    for g in range(2):
        for h in range(8):
            for p in range(P):
                for hh in range(4):
                    if h == 4 * g + hh:
                        s32[g, h * 4 + p, hh * 32:(hh + 1) * 32] = 1.0
    for r in range(4):
        for g in range(2):
            sel[r * 32:(r + 1) * 32, g * 128:(g + 1) * 128] = s32[g]
    for g in range(2):
        sel[96:128, 256 + g * 128:256 + (g + 1) * 128] = s32[g]
    selp32 = np.zeros((32, 32), np.float32)
    for h in range(8):
        selp32[h * 4:(h + 1) * 4, h * 4:(h + 1) * 4] = 1.0
    selp = np.kron(np.eye(4, dtype=np.float32), selp32)
    iden = np.eye(128, dtype=np.float32)
    xm = np.ones((128, 2 * RB), np.float32)
    j = np.arange(RB)
    xm[:, 0:RB] = ((j % W) < W - 1).astype(np.float32)[None, :]
    xm[:, RB:2 * RB] = ((j % W) > 0).astype(np.float32)[None, :]
    return (sel.astype(bf), selp.astype(bf), iden.astype(bf), xm.astype(bf))


def prep_inputs(x, w_off, b_off, w_attn, b_attn, w_val, b_val, w_out, b_out):
    import ml_dtypes
    bf = ml_dtypes.bfloat16
    sel, selp, iden, xmask = _consts()
    woa = np.concatenate([w_off[1::2], w_off[0::2], w_attn], 0).T
    by = np.zeros((128, 5), np.float32)
    bx = np.zeros((128, 5), np.float32)
    ba = np.zeros((128, 1), np.float32)
    for r in range(4):
        for k, s in enumerate(SYS):
            by[r * 32:(r + 1) * 32, k] = b_off[1::2] - s
            bx[r * 32:(r + 1) * 32, k] = b_off[0::2] - s
        ba[r * 32:(r + 1) * 32, 0] = b_attn
    shared = {
        "wv": np.ascontiguousarray(w_val.T).astype(bf),
        "woa": np.ascontiguousarray(woa).astype(bf),
        "wo": np.ascontiguousarray(w_out.T).astype(bf),
        "sel": sel, "selp": selp, "iden": iden, "xmask": xmask,
        "bval": np.ascontiguousarray(b_val[:, None]).astype(np.float32),
        "bout": np.ascontiguousarray(b_out[:, None]).astype(np.float32),
        "biasy": by, "biasx": bx, "boaa": ba,
    }
    in_maps = []
    for core in range(8):
        b, j = divmod(core, 4)
        n0 = j * NB
        xb = np.asarray(x[b], np.float32).reshape(C, N)
        xt = np.zeros((C, XEXT), np.float32)
        lo, hi = n0 - HALO, n0 + NB + HALO
        clo, chi = max(lo, 0), min(hi, N)
        xt[:, clo - lo:chi - lo] = xb[:, clo:chi]
        vm = np.ones((128, 2 * HALO), np.float32)
        if j == 0:
            vm[:, :HALO] = 0.0
        if j == 3:
            vm[:, HALO:] = 0.0
        m = dict(shared)
        m["xT"] = xt.astype(bf)
        m["vmask"] = vm.astype(bf)
        in_maps.append(m)
    return in_maps


def assemble(results):
    out = np.zeros((B, C, N), np.float32)
    for core in range(8):
        b, j = divmod(core, 4)
        out[b, :, j * NB:(j + 1) * NB] = np.asarray(
            results[core]["out"], np.float32)
    return out.reshape(B, C, Z, HH, WW)


last_exec_ns = None


def kernel(**inputs):
    global last_exec_ns
    from concourse.bass_utils import run_bass_kernel_spmd
    nc = build_program()
    in_maps = prep_inputs(**inputs)
    res = run_bass_kernel_spmd(nc, in_maps, list(range(8)))
    last_exec_ns = res.exec_time_ns
    return assemble(res.results)
